# revision 11
# baseline (speedup 1.0000x reference)
"""Trainium2 Bass kernel for nn_CustomLoss_47931835023913.

Computes: loss = mean_i( logsumexp(output[i,:]) - output[i, target[i]] )
                 + (epoch**-0.65)*64 + 0.01   if any(target==2 & argmax==3)

Strategy (data-parallel over 8 NeuronCores, batch-sharded):
  * Host-side layout prep only: each row of `output` is rotated so that
    column 0 holds output[i, target[i]] (the CE gather becomes a strided
    column sum) and, for rows with target==2, column 1 holds output[i, 3]
    (the argmax flag test becomes a compare against the row max, which is
    rotation-invariant). A uint8 mask of target==2 rides along. Inputs ship
    as bf16 (CE mean over 4.2M rows is insensitive to unbiased rounding).
    All O(B) loss arithmetic runs on the NeuronCores.
  * Device per 128x512x10 tile:
      - ScalarE: exp() into two bf16 half-tiles (classes 0-4 / 5-9).
      - TensorE: 10 accumulating identity matmuls sum the halves' columns
        into PSUM -> per-row sum(exp) in fp32 (partition-passthrough adds).
      - ScalarE: ln() of the PSUM row sums with accum_out -> per-tile
        partial sum of the logsumexp term; a strided Identity accum over
        rotated column 0 -> partial sum of gathered logits.
      - VectorE: pairwise-tree max (bf16 tensor_tensor, 2x mode) for the
        row max; flag partial = sum(mask * (e[:,1] >= rowmax)).
  * Host combines the 8 cores' [128, 3*T] accumulators in float64 and adds
    the epoch correction.

bf16 exp values only feed (a) ln(sum(exp)) -- unbiased rounding noise that
averages out over 4.2M rows -- and (b) the argmax compare, where round-to-
nearest monotonicity guarantees no false negatives on the any() flag.
"""

import numpy as np

B = 4194304          # batch rows
C = 10               # classes
NCORES = 8
P = 128              # SBUF partitions
R = B // NCORES      # rows per core            = 524288
RP = R // P          # rows per partition       = 4096
TN = 512             # tile rows per partition
T = RP // TN         # tiles per core           = 8
MMN = 512            # matmul free-dim slice (one PSUM bank)

_CACHE = {}

# all activation funcs this kernel uses live in this one table set, so pin
# every InstActivation to it -> exactly one LoadActFuncSet in the program
_ACT_SET = "natural_log_exp_and_others"


def _pin_act_tables():
    import concourse.bacc as bacc_mod

    if getattr(bacc_mod.get_activation_tables, "_pinned", False):
        return
    orig = bacc_mod.get_activation_tables

    def pinned(module_arch):
        tables = orig(module_arch)
        return {
            name: (funcs if name == _ACT_SET else set())
            for name, funcs in tables.items()
        }

    pinned._pinned = True
    bacc_mod.get_activation_tables = pinned


def _build_nc(repeat=1):
    import concourse.mybir as mybir
    from concourse.bacc import Bacc
    from concourse.tile import TileContext

    _pin_act_tables()

    A = mybir.AluOpType
    F = mybir.ActivationFunctionType
    f32 = mybir.dt.float32
    bf16 = mybir.dt.bfloat16

    nc = Bacc("TRN2")
    x_d = nc.dram_tensor("x", [P, RP * C], bf16, kind="ExternalInput")
    m2_d = nc.dram_tensor("m2", [P, RP], mybir.dt.uint8, kind="ExternalInput")
    out_d = nc.dram_tensor("out", [P, 3 * T], f32, kind="ExternalOutput")
    ident_d = nc.inline_tensor(
        np.eye(P, dtype=np.float32), name="ident"
    )  # cast to bf16 on load

    with TileContext(nc) as tc:
        with (
            tc.tile_pool(name="persist", bufs=1) as pp,
            tc.tile_pool(name="io", bufs=4) as iop,
            tc.tile_pool(name="work", bufs=3) as wp,
            tc.tile_pool(name="ps", bufs=4, space="PSUM") as psp,
        ):
            m2_all = pp.tile([P, RP], mybir.dt.uint8)
            nc.sync.dma_start(m2_all[:], m2_d[:])
            ident_f = pp.tile([P, P], f32)
            nc.sync.dma_start(ident_f[:], ident_d[:])
            ident = pp.tile([P, P], bf16)
            nc.vector.tensor_copy(ident[:], ident_f[:])
            acc = pp.tile([P, 3 * T], f32)

            for k in range(T * repeat):
                k = k % T
                x_t = iop.tile([P, TN * C], bf16, tag="x")
                nc.sync.dma_start(x_t[:], x_d[:, k * TN * C : (k + 1) * TN * C])
                xv = x_t.rearrange("p (n c) -> p n c", c=C)

                e_all = wp.tile([P, TN * C], bf16, tag="e")
                ev = e_all.rearrange("p (n c) -> p n c", c=C)
                nc.scalar.activation(ev, xv, F.Exp)

                # row sum of the 10 exp columns: accumulating identity
                # matmuls (partition passthrough), one PSUM bank per
                # 512-column group
                s_ps = psp.tile([P, TN], f32, tag="s")
                for g in range(TN // MMN):
                    rows = slice(g * MMN, (g + 1) * MMN)
                    for c in range(C):
                        nc.tensor.matmul(
                            s_ps[:, rows], ident[:], ev[:, rows, c],
                            start=(c == 0), stop=(c == C - 1),
                        )

                # row max: pairwise tree on bf16 (2x mode on level 1)
                mx1 = wp.tile([P, TN * 5], bf16, tag="mx1")
                mx1v = mx1.rearrange("p (n c) -> p n c", c=5)
                nc.vector.tensor_tensor(mx1v, ev[:, :, 0:5], ev[:, :, 5:10], A.max)
                mx2 = wp.tile([P, TN * 2], bf16, tag="mx2")
                mx2v = mx2.rearrange("p (n c) -> p n c", c=2)
                nc.vector.tensor_tensor(mx2v, mx1v[:, :, 0:2], mx1v[:, :, 2:4], A.max)
                v = wp.tile([P, TN], f32, tag="v")
                nc.vector.tensor_tensor(v[:], mx2v[:, :, 0], mx2v[:, :, 1], A.max)
                rmax = wp.tile([P, TN], f32, tag="rmax")
                nc.vector.tensor_tensor(rmax[:], v[:], mx1v[:, :, 4], A.max)

                # partial sums: lse (ACT), gathered logit (DVE), flag (DVE)
                lse_scr = wp.tile([P, TN], f32, tag="lse_scr")
                nc.scalar.activation(
                    lse_scr[:], s_ps[:], F.Ln, accum_out=acc[:, k : k + 1]
                )
                g_scr = wp.tile([P, TN], f32, tag="g_scr")
                nc.vector.tensor_scalar(
                    g_scr[:], xv[:, :, 0], 1.0, 0.0, A.mult, A.add,
                    accum_out=acc[:, T + k : T + k + 1],
                )
                eq = wp.tile([P, TN], f32, tag="eq")
                nc.vector.tensor_tensor(eq[:], ev[:, :, 1], rmax[:], A.is_ge)
                f_scr = wp.tile([P, TN], f32, tag="f_scr")
                nc.vector.scalar_tensor_tensor(
                    f_scr[:], m2_all[:, k * TN : (k + 1) * TN], 1.0, eq[:],
                    A.mult, A.mult, accum_out=acc[:, 2 * T + k : 2 * T + k + 1],
                )

            nc.sync.dma_start(out_d[:], acc[:])
    nc.finalize()
    return nc


def _get_nc():
    if "nc" not in _CACHE:
        _CACHE["nc"] = _build_nc()
    return _CACHE["nc"]


def _prep_inputs(x, t32):
    """Rotate each row so column 0 is the target logit; build target==2 mask."""
    import ml_dtypes

    idx = (t32[:, None] + np.arange(C, dtype=np.int32)[None, :]) % C
    xr = np.take_along_axis(x, idx, axis=1).astype(ml_dtypes.bfloat16)
    m2 = (t32 == 2).astype(np.uint8)
    xs = xr.reshape(NCORES, P, RP * C)
    ms = m2.reshape(NCORES, P, RP)
    return xs, ms


def kernel(output=None, target=None, epoch=None):
    from concourse import bass_utils

    x = np.asarray(output)
    if x.dtype != np.float32:
        x = x.astype(np.float32)
    t32 = np.asarray(target).astype(np.int32)
    ep = int(np.asarray(epoch))
    assert x.shape == (B, C) and t32.shape == (B,)

    xs, ms = _prep_inputs(x, t32)
    in_maps = [
        {"x": np.ascontiguousarray(xs[i]), "m2": np.ascontiguousarray(ms[i])}
        for i in range(NCORES)
    ]
    nc = _get_nc()
    res = bass_utils.run_bass_kernel_spmd(nc, in_maps, core_ids=list(range(NCORES)))

    lse_sum = 0.0
    g_sum = 0.0
    flg = 0.0
    for rmap in res.results:
        o = rmap["out"].astype(np.float64)
        lse_sum += o[:, 0:T].sum()
        g_sum += o[:, T : 2 * T].sum()
        flg += o[:, 2 * T : 3 * T].sum()

    init_loss = (lse_sum - g_sum) / B
    corr = (float(ep) ** -0.65) / (4.0 ** -3) + 0.01
    loss = init_loss + (corr if flg > 0 else 0.0)
    return np.array(loss, dtype=np.float32)
